# revision 14
# baseline (speedup 1.0000x reference)
"""BiLSTM-CRF forward-scoring kernel on 8 Trainium2 NeuronCores (Bass/Tile).

Model (hardcoded): V=50000, E=256, H=256/dir (H2=512), T=16 tags, B=32,
S=512, START=14, STOP=15.

Device strategy:
  - 8 cores = 4 batch groups x 2 LSTM directions. Core c in 0..3 runs the
    FORWARD LSTM for sequences [8c, 8c+8); core c+4 runs the BACKWARD LSTM
    for the same sequences (on host-reversed input - identical program).
  - Each core: embedding rows arrive pre-gathered (host numpy gather),
    input GEMM (PE, bf16), 512-step LSTM recurrence (gates on partitions),
    half-emissions GEMM, pairwise AllReduce to form full emissions,
    then HALF of the CRF chain in the exp-domain (forward alpha chain on
    fwd cores, suffix/beta chain on bwd cores; 256 steps each), pairwise
    AllGather + combine -> log_Z per sequence.
  - CRF steps run entirely in the exp domain with per-step max
    renormalization (scales accumulated in log space at the end), so the
    inner loop is 1 tiny matmul + 3 DVE ops, no transcendentals.
"""

import sys

sys.path.insert(0, "/opt/trn_rl_repo")

import numpy as np
import ml_dtypes

import concourse.bass as bass
from concourse import bacc
import concourse.mybir as mybir
from concourse.tile import TileContext
from concourse.masks import make_identity
from concourse.bass_utils import run_bass_kernel_spmd

V, E, H, T, B, S_FULL = 50000, 256, 256, 16, 32, 512
G = 4 * H  # 1024 gates per direction
START, STOP = 14, 15
NC = 8
BL = 8  # sequences per core
PAIRS = [[0, 4], [1, 5], [2, 6], [3, 7]]

f32 = mybir.dt.float32
bf16 = mybir.dt.bfloat16
i32 = mybir.dt.int32

AF = mybir.ActivationFunctionType
ALU = mybir.AluOpType


# --------------------------------------------------------------------------
# device program
# --------------------------------------------------------------------------
def build_program(S: int) -> bass.Bass:
    nc = bacc.Bacc("TRN2", target_bir_lowering=False, debug=False, num_devices=NC)

    xT_d = nc.declare_dram_parameter("xT", [128, 2, S * BL], bf16, isOutput=False)
    WiT_d = nc.declare_dram_parameter("WiT", [128, 2 * 8 * 128], bf16, isOutput=False)
    WhT_d = nc.declare_dram_parameter("WhT", [128, 2 * 8 * 128], bf16, isOutput=False)
    bias_d = nc.declare_dram_parameter("bias", [128, 8], f32, isOutput=False)
    WtT_d = nc.declare_dram_parameter("WtT", [128, 2 * T], bf16, isOutput=False)
    bth_d = nc.declare_dram_parameter("bth", [T, 1], f32, isOutput=False)
    R_d = nc.declare_dram_parameter("R", [T, T], f32, isOutput=False)
    ea0_d = nc.declare_dram_parameter("ea0", [T, BL], f32, isOutput=False)
    msk_d = nc.declare_dram_parameter("msk", [T, 3], f32, isOutput=False)
    out_d = nc.declare_dram_parameter("logz", [BL, 1], f32, isOutput=True)

    E_loc = nc.dram_tensor("E_loc", [T, S, BL], f32)
    E_red = nc.dram_tensor("E_red", [T, S, BL], f32)
    P_loc = nc.dram_tensor("P_loc", [BL, T + 1], f32)
    P_all = nc.dram_tensor("P_all", [2, BL, T + 1], f32)

    CB = min(512, S * BL)  # GEMM column-block size
    NB = (S * BL) // CB  # input-GEMM column blocks
    SC = S // 2  # CRF steps per half-chain
    TB = S // 16  # eemit transpose blocks (16 timesteps each)

    with TileContext(nc) as tc:
        with (
            tc.tile_pool(name="const", bufs=1) as cpool,
            tc.tile_pool(name="slab", bufs=1) as slab,
            tc.tile_pool(name="work", bufs=3) as work,
            tc.tile_pool(name="psA", bufs=4, space="PSUM") as psA,
        ):
            # ---- constants / inputs to SBUF
            WiT = cpool.tile([128, 2, 8, 128], bf16)
            WhT = cpool.tile([128, 2, 8, 128], bf16)
            bias = cpool.tile([128, 8], f32)
            WtT = cpool.tile([128, 2, T], bf16)
            bth = cpool.tile([T, 1], f32)
            Rm = cpool.tile([T, T], f32)
            ea0 = cpool.tile([T, BL], f32)
            msk = cpool.tile([T, 3], f32)

            nc.sync.dma_start(out=WiT[:].rearrange("p a b c -> p (a b c)"), in_=WiT_d[:])
            nc.sync.dma_start(out=WhT[:].rearrange("p a b c -> p (a b c)"), in_=WhT_d[:])
            nc.sync.dma_start(out=bias[:], in_=bias_d[:])
            nc.sync.dma_start(out=WtT[:].rearrange("p a b -> p (a b)"), in_=WtT_d[:])
            nc.sync.dma_start(out=bth[:], in_=bth_d[:])
            nc.sync.dma_start(out=Rm[:], in_=R_d[:])
            nc.sync.dma_start(out=ea0[:], in_=ea0_d[:])
            nc.sync.dma_start(out=msk[:], in_=msk_d[:])

            # ---- phase B: input GEMM  pre[g, t, b] = Wi @ x + (bi + bh)
            pp = tc.alloc_tile_pool(name="pp", bufs=1)
            px = tc.alloc_tile_pool(name="px", bufs=1)
            xT = px.tile([128, 2, S * BL], bf16)
            nc.sync.dma_start(out=xT[:].rearrange("p a b -> p (a b)"), in_=xT_d[:])
            preT = pp.tile([128, 8, S * BL], bf16)
            for nb in range(NB):
                for gc in range(8):
                    zp = psA.tile([128, CB], f32, tag="ps")
                    for kc in range(2):
                        nc.tensor.matmul(
                            zp[:],
                            WiT[:, kc, gc, :],
                            xT[:, kc, nb * CB : (nb + 1) * CB],
                            start=(kc == 0),
                            stop=(kc == 1),
                        )
                    dst = preT[:, gc, nb * CB : (nb + 1) * CB]
                    if gc % 2 == 0:
                        nc.vector.tensor_scalar_add(dst, zp[:], bias[:, gc : gc + 1])
                    else:
                        nc.scalar.activation(
                            dst, zp[:], AF.Identity, bias=bias[:, gc : gc + 1]
                        )

            px.release()

            # ---- phase C: LSTM recurrence (gates on partitions)
            # hs[p, t, hc, b]: slot 0 is h_{-1}=0; step t writes slot t+1.
            hs = slab.tile([128, S + 1, 2, BL], bf16)
            c_sb = slab.tile([128, 2, BL], f32)
            nc.vector.memset(hs[:, 0, :, :], 0.0)
            nc.vector.memset(c_sb[:], 0.0)
            for t in range(S):
                zp = psA.tile([128, 8, BL], f32, tag="ps")
                for gc in range(8):
                    for kc in range(2):
                        nc.tensor.matmul(
                            zp[:, gc, :],
                            WhT[:, kc, gc, :],
                            hs[:, t, kc, :],
                            start=(kc == 0),
                            stop=(kc == 1),
                        )
                nc.vector.tensor_add(zp[:], zp[:], preT[:, :, t * BL : (t + 1) * BL])
                gts = work.tile([128, 6, BL], f32, tag="gts")
                nc.scalar.activation(gts[:], zp[:, 0:6, :], AF.Sigmoid)
                tg = work.tile([128, 2, BL], f32, tag="tg")
                nc.scalar.activation(tg[:], zp[:, 6:8, :], AF.Tanh)
                t1 = work.tile([128, 2, BL], f32, tag="t1")
                nc.vector.tensor_mul(t1[:], gts[:, 0:2, :], tg[:])
                nc.vector.tensor_mul(c_sb[:], gts[:, 2:4, :], c_sb[:])
                nc.vector.tensor_add(c_sb[:], c_sb[:], t1[:])
                th = work.tile([128, 2, BL], f32, tag="th")
                nc.scalar.activation(th[:], c_sb[:], AF.Tanh)
                nc.vector.tensor_mul(hs[:, t + 1, :, :], gts[:, 4:6, :], th[:])

            pp.release()

            # ---- phase D: half-emissions GEMM + pairwise exchange
            pe = tc.alloc_tile_pool(name="pe", bufs=1)
            E_sb = pe.tile([T, S, BL], f32, tag="Ee")
            for nb in range(NB):
                t0, t1b = nb * (S // NB), (nb + 1) * (S // NB)
                ep = psA.tile([T, CB], f32, tag="ps")
                for kc in range(2):
                    nc.tensor.matmul(
                        ep[:],
                        WtT[:, kc, :],
                        hs[:, 1 + t0 : 1 + t1b, kc, :],
                        start=(kc == 0),
                        stop=(kc == 1),
                    )
                nc.scalar.activation(
                    E_sb[:, t0:t1b, :].rearrange("p a b -> p (a b)"),
                    ep[:],
                    AF.Identity,
                    bias=bth[:],
                )

            Erev = pe.tile([T, S, BL], f32, tag="Er")
            nc.sync.dma_start(out=Erev[:], in_=E_sb[:, S - 1 :: -1, :])
            nc.vector.tensor_scalar(
                Erev[:], Erev[:], msk[:, 1:2], None, ALU.mult
            )
            nc.vector.scalar_tensor_tensor(
                Erev[:], E_sb[:], msk[:, 0:1], Erev[:], ALU.mult, ALU.add
            )
            nc.sync.dma_start(out=E_loc[:], in_=Erev[:])

            nc.gpsimd.collective_compute(
                "AllReduce",
                ALU.add,
                replica_groups=PAIRS,
                ins=[E_loc[:]],
                outs=[E_red[:]],
            )

            # Epad[p, t, c]: tag n on partitions 0..15 pre-transpose;
            # per-t 32-col bands so DVE 32x32 block-transpose yields
            # eemitT[b, t, n] with b on partitions 0..7.
            Epad = pe.tile([32, S, 32], f32)
            eemitT = slab.tile([32, S, 32], f32)
            initE = cpool.tile([T, BL], f32)
            nc.vector.memset(Epad[:], 0.0)
            # alpha core (m0=1): slab[j] = E[j], initE = 0 (exp(0)=1).
            # sigma core (m1=1): slab[j] = E[S-2-j] (j<S-1), slab[SC-1]=0,
            # initE = E[S-1].
            Eldn = pe.tile([T, S, BL], f32, tag="Ee")
            Eldr = pe.tile([T, S, BL], f32, tag="Er")
            nc.sync.dma_start(out=Eldn[:], in_=E_red[:])
            nc.vector.memset(Eldr[:, S - 1, :], 0.0)
            nc.sync.dma_start(
                out=Eldr[:, 0 : S - 1, :], in_=E_red[:, S - 2 :: -1, :]
            )
            nc.vector.tensor_scalar(
                Eldr[:], Eldr[:], msk[:, 1:2], None, ALU.mult
            )
            nc.vector.scalar_tensor_tensor(
                Epad[0:T, :, 0:BL], Eldn[:], msk[:, 0:1], Eldr[:],
                ALU.mult, ALU.add,
            )
            nc.vector.tensor_scalar(
                Epad[0:T, SC - 1, 0:BL], Epad[0:T, SC - 1, 0:BL],
                msk[:, 0:1], None, ALU.mult,
            )
            itmp = work.tile([T, BL], f32, tag="itmp")
            nc.sync.dma_start(out=itmp[:], in_=E_red[:, S - 1, :])
            nc.vector.tensor_scalar(
                initE[:], itmp[:], msk[:, 1:2], None, ALU.mult
            )

            nc.scalar.activation(Epad[0:T, :, 0:BL], Epad[0:T, :, 0:BL], AF.Exp)
            for m in range(S // 16):
                nc.vector.transpose(
                    eemitT[:, m * 16 : (m + 1) * 16, :],
                    Epad[:, m * 16 : (m + 1) * 16, :],
                )

            pe.release()

            # ---- phase E: CRF half-chain, exp domain
            ea_T = slab.tile([32, 32], f32)  # [prev, b] in [0:16, 0:8]
            ea_st = slab.tile([32, 32], f32)  # [b, next] in [0:8, 0:16]
            m_slab = slab.tile([BL, SC], f32)
            nc.vector.memset(ea_T[:], 0.0)
            nc.vector.memset(ea_st[:], 0.0)
            einit = work.tile([T, BL], f32, tag="einit")
            nc.scalar.activation(einit[:], initE[:], AF.Exp)
            nc.vector.tensor_mul(ea_T[0:T, 0:BL], ea0[:], einit[:])

            for i in range(SC):
                pm = psA.tile([BL, T], f32, tag="ps")
                nc.tensor.matmul(
                    pm[:], ea_T[0:T, 0:BL], Rm[:], start=True, stop=True
                )
                nc.vector.tensor_mul(
                    ea_st[0:BL, 0:T], pm[:], eemitT[0:BL, i, 0:T]
                )
                nc.vector.tensor_reduce(
                    m_slab[:, i : i + 1], ea_st[0:BL, 0:T],
                    mybir.AxisListType.X, ALU.max,
                )
                rm = work.tile([BL, 1], f32, tag="rm")
                nc.vector.reciprocal(rm[:], m_slab[:, i : i + 1])
                nc.vector.tensor_scalar_mul(
                    ea_st[0:BL, 0:T], ea_st[0:BL, 0:T], rm[:]
                )
                nc.vector.transpose(ea_T[:], ea_st[:])

            # ---- final combine
            mlog = work.tile([BL, SC], f32, tag="mlog")
            nc.scalar.activation(mlog[:], m_slab[:], AF.Ln)
            lsum = work.tile([BL, 1], f32, tag="lsum")
            nc.vector.tensor_reduce(lsum[:], mlog[:], mybir.AxisListType.X, ALU.add)
            pack = work.tile([BL, T + 1], f32, tag="pack")
            nc.vector.tensor_copy(pack[:, 0:T], ea_st[0:BL, 0:T])
            nc.vector.tensor_copy(pack[:, T : T + 1], lsum[:])
            nc.sync.dma_start(out=P_loc[:], in_=pack[:])

            nc.gpsimd.collective_compute(
                "AllGather",
                ALU.bypass,
                replica_groups=PAIRS,
                ins=[P_loc[:]],
                outs=[P_all[:]],
            )

            pf = work.tile([BL, T + 1], f32, tag="pf")
            pb = work.tile([BL, T + 1], f32, tag="pb")
            nc.sync.dma_start(out=pf[:], in_=P_all[0])
            nc.sync.dma_start(out=pb[:], in_=P_all[1])
            junk = work.tile([BL, T], f32, tag="junk")
            ssum = work.tile([BL, 1], f32, tag="ssum")
            nc.vector.tensor_mul(junk[:], pf[:, 0:T], pb[:, 0:T])
            nc.vector.tensor_reduce(
                ssum[:], junk[:], mybir.AxisListType.X, ALU.add
            )
            lz = work.tile([BL, 1], f32, tag="lz")
            nc.scalar.activation(lz[:], ssum[:], AF.Ln)
            nc.vector.tensor_add(lz[:], lz[:], pf[:, T : T + 1])
            nc.vector.tensor_add(lz[:], lz[:], pb[:, T : T + 1])
            nc.sync.dma_start(out=out_d[:], in_=lz[:])

    nc.finalize()
    return nc


# --------------------------------------------------------------------------
# host-side data prep
# --------------------------------------------------------------------------
def _arrange_lhsT(W):
    """W [G=1024(out), K=256(in)] -> lhsT tiles [128, kc=2, gc=8, m=128] where
    lhsT[p, kc, gc, m] = W[gc*128+m, kc*128+p]."""
    Wr = W.reshape(8, 128, 2, 128)  # [gc, m, kc, p]
    return np.ascontiguousarray(Wr.transpose(3, 2, 0, 1))  # [p, kc, gc, m]


def _gate_perm(Wrows):
    """Reorder gate rows from [i,f,g,o] to [i,f,o,g] blocks of H."""
    i, f, g, o = (Wrows[k * H : (k + 1) * H] for k in range(4))
    return np.concatenate([i, f, o, g], axis=0)


def prepare_in_maps(tokens, embed_table, Wi_f, Wh_f, bi_f, bh_f,
                    Wi_b, Wh_b, bi_b, bh_b, Wt, bt, transitions, S=S_FULL):
    tokens = np.asarray(tokens)
    x = np.asarray(embed_table, np.float32)[tokens]  # [B, S, E]
    trans = np.asarray(transitions, np.float32)
    Rexp = np.exp(trans)  # exp(-1e4) == 0

    in_maps = []
    for core in range(NC):
        fwd = core < 4
        grp = core % 4
        xg = x[grp * BL : (grp + 1) * BL, :S]  # [BL, S, E]
        if not fwd:
            xg = xg[:, ::-1]
        # xT[p, kc, t*BL+b] = xg[b, t, kc*128+p]
        xT = np.ascontiguousarray(
            xg.reshape(BL, S, 2, 128).transpose(3, 2, 1, 0)
        ).reshape(128, 2, S * BL)

        Wi, Wh = (Wi_f, Wh_f) if fwd else (Wi_b, Wh_b)
        bsum = (bi_f + bh_f) if fwd else (bi_b + bh_b)
        Wi = _gate_perm(np.asarray(Wi, np.float32))
        Wh = _gate_perm(np.asarray(Wh, np.float32))
        bsum = _gate_perm(np.asarray(bsum, np.float32).reshape(G, 1))[:, 0]
        bias = np.ascontiguousarray(bsum.reshape(8, 128).T)  # [128, 8]

        Wt_half = np.asarray(Wt, np.float32)[:, 0:H] if fwd else \
            np.asarray(Wt, np.float32)[:, H : 2 * H]
        # WtT[p, kc, n] = Wt_half[n, kc*128+p]
        WtT = np.ascontiguousarray(
            Wt_half.reshape(T, 2, 128).transpose(2, 1, 0)
        ).reshape(128, 2 * T)
        bth = (np.asarray(bt, np.float32) if fwd else np.zeros(T, np.float32))

        if fwd:
            R = np.ascontiguousarray(Rexp.T)  # R[prev, next] = exp(trans[n,p])
            ea0 = np.zeros((T, BL), np.float32)
            ea0[START, :] = 1.0
        else:
            R = np.ascontiguousarray(Rexp)  # R[k, j] = exp(trans[k, j])
            ea0 = np.tile(Rexp[STOP, :][:, None], (1, BL)).astype(np.float32)

        in_maps.append({
            "xT": xT.astype(ml_dtypes.bfloat16),
            "WiT": _arrange_lhsT(Wi).reshape(128, -1).astype(ml_dtypes.bfloat16),
            "WhT": _arrange_lhsT(Wh).reshape(128, -1).astype(ml_dtypes.bfloat16),
            "bias": bias,
            "WtT": WtT.astype(ml_dtypes.bfloat16),
            "bth": bth.reshape(T, 1),
            "R": R,
            "ea0": ea0,
            "msk": np.tile(np.array(
                [[1.0, 0.0, 1.0] if fwd else [0.0, 1.0, 0.0]], np.float32),
                (T, 1)),
        })
    return in_maps


_prog_cache = {}


def _get_prog(S):
    if S not in _prog_cache:
        _prog_cache[S] = build_program(S)
    return _prog_cache[S]


def kernel(tokens, embed_table, Wi_f, Wh_f, bi_f, bh_f,
           Wi_b, Wh_b, bi_b, bh_b, Wt, bt, transitions):
    in_maps = prepare_in_maps(tokens, embed_table, Wi_f, Wh_f, bi_f, bh_f,
                              Wi_b, Wh_b, bi_b, bh_b, Wt, bt, transitions)
    nc = _get_prog(S_FULL)
    res = run_bass_kernel_spmd(nc, in_maps, list(range(NC)))
    outs = [np.asarray(res.results[c]["logz"], np.float32).reshape(BL)
            for c in range(4)]
    return np.concatenate(outs).astype(np.float32)
